# revision 2
# baseline (speedup 1.0000x reference)
"""DLinear (causal-window decomposition + dual Linear) as a single fused matmul
on 8 Trainium2 NeuronCores.

Algebra: with A the [T,T] causal-window-mean operator (banded, window=25),
    out = x @ (Sw + (Tw - Sw) @ A)^T + (tb + sb)
A is banded, so the fold happens on the host in O(T^2); the device runs one
[2048,721] x [721,720] matmul per core with the bias riding as an extra
contraction row against a ones row appended to x^T.

Device schedule (v2): W-stationary.  lhsT = W k-chunk [kc<=128, u<=128]
(stationary), rhs = x k-chunk [kc, 512] (moving), PSUM out [u, 512] fp32
accumulated over the 6 k-chunks.  144 matmuls of N=512 amortize the per-matmul
LDWEIGHTS better than the previous 192 of N=360.  Inputs stream fine-grained
(x per 512-column block on the sync ring, W per 360-column half on the scalar
ring) so real matmuls start during the DMA ramp; per-unit stores spread the
output across the whole kernel instead of a serial tail.  Output leaves the
device transposed [720, 2048] (1KB contiguous lines); the host transposes
back.  Data moves as fp16 with fp32 PSUM accumulation (~4e-4 rel err).
"""

import sys
import types

import numpy as np

import concourse.bacc as bacc
import concourse.mybir as mybir
from concourse import tile
from concourse.bass_utils import run_bass_kernel_spmd

# bass_utils imports antenv.axon_hooks when tracing is requested; some images
# lack that module.  Provide a no-op shim so the run degrades to untraced.
try:
    import antenv.axon_hooks  # noqa: F401
except ImportError:
    try:
        import antenv
        _shim = types.ModuleType("antenv.axon_hooks")
        _shim._hook = None
        _shim.set_axon_ntff_profile_hook = lambda h: setattr(_shim, "_hook", h)
        _shim.get_axon_ntff_profile_hook = lambda: _shim._hook
        sys.modules["antenv.axon_hooks"] = _shim
        antenv.axon_hooks = _shim
    except ImportError:
        pass

WINDOW = 25
B, NPTS, T = 32, 512, 720
U = T                     # output features
N_CORES = 8
M_TOT = B * NPTS          # 16384 rows
M_LOC = M_TOT // N_CORES  # 2048 rows per core
P = 128                   # partitions
KE = T + 1                # contraction incl. bias row = 721
KFULL = KE // P           # 5 full 128-row k-chunks
KREM = KE - KFULL * P     # 81 rows in the last chunk
NKC = KFULL + 1           # 6 k-chunks
MC_W = 512                # moving free dim per matmul (PSUM bank = 512 fp32)
N_MC = M_LOC // MC_W      # 4 column blocks of x
U_CHUNKS = [(0, 128), (128, 128), (256, 128), (384, 128), (512, 128),
            (640, 80)]

_F32 = mybir.dt.float32
_F16 = mybir.dt.float16
N_WARMUP = 3              # junk matmuls to start the PE HAM clock early


def _build_nc():
    nc = bacc.Bacc("TRN2", target_bir_lowering=False, debug=False,
                   num_devices=N_CORES, enable_partition_id=False)
    xt_d = nc.dram_tensor("xt", [KE, M_LOC], _F16, kind="ExternalInput").ap()
    wt_d = nc.dram_tensor("wt", [KE, U], _F16, kind="ExternalInput").ap()
    out_d = nc.dram_tensor("out", [U, M_LOC], _F16, kind="ExternalOutput").ap()
    xt_main = xt_d[0:KFULL * P, :].rearrange("(k p) m -> p k m", p=P)
    xt_rem = xt_d[KFULL * P:KE, :]
    wt_main = wt_d[0:KFULL * P, :].rearrange("(k p) u -> p k u", p=P)
    wt_rem = wt_d[KFULL * P:KE, :]

    with tile.TileContext(nc) as tc:
        with tc.tile_pool(name="wpool", bufs=1) as wpool, \
             tc.tile_pool(name="xpool", bufs=1) as xpool, \
             tc.tile_pool(name="opool", bufs=4) as opool, \
             tc.tile_pool(name="jp", bufs=1, space="PSUM") as jp, \
             tc.tile_pool(name="accp", bufs=4, space="PSUM") as accp:

            # Junk matmuls: start the HAM activity window while the first
            # DMAs land.
            scr = wpool.tile([P, 640], _F16, name="scr", tag="scr")
            nc.gpsimd.memset(scr[:], 0.0)
            ps_scr = jp.tile([P, 512], _F32, name="ps_scr", tag="ps_scr")
            for _ in range(N_WARMUP):
                nc.tensor.matmul(ps_scr[:], scr[:, 0:P], scr[:, P:P + 512],
                                 start=True, stop=True)

            w_all = wpool.tile([P, NKC * U], _F16, name="w_all", tag="w_all")
            w_v = w_all[:].rearrange("p (k u) -> p k u", k=NKC)
            x_all = xpool.tile([P, NKC * M_LOC], _F16, name="x_all",
                               tag="x_all")
            x_v = x_all[:].rearrange("p (k m) -> p k m", k=NKC)

            # W halves on the scalar ring (shared later with stores), x column
            # blocks on the sync ring; the two rings drain in parallel.
            nc.scalar.dma_start(w_v[:, 0:KFULL, 0:360], wt_main[:, :, 0:360])
            nc.scalar.dma_start(w_v[0:KREM, KFULL, 0:360], wt_rem[:, 0:360])
            nc.scalar.dma_start(w_v[:, 0:KFULL, 360:720],
                                wt_main[:, :, 360:720])
            nc.scalar.dma_start(w_v[0:KREM, KFULL, 360:720],
                                wt_rem[:, 360:720])
            for mc in range(N_MC):
                ms = slice(mc * MC_W, (mc + 1) * MC_W)
                nc.sync.dma_start(x_v[:, 0:KFULL, ms], xt_main[:, :, ms])
                nc.sync.dma_start(x_v[0:KREM, KFULL, ms], xt_rem[:, ms])

            for mc in range(N_MC):
                ms = slice(mc * MC_W, (mc + 1) * MC_W)
                for ui, (u0, uw) in enumerate(U_CHUNKS):
                    acc = accp.tile([P, MC_W], _F32, name="acc", tag="acc")
                    for k in range(NKC):
                        kc = P if k < KFULL else KREM
                        nc.tensor.matmul(acc[0:uw, :],
                                         w_v[0:kc, k, u0:u0 + uw],
                                         x_v[0:kc, k, ms],
                                         start=(k == 0), stop=(k == NKC - 1))
                    ot = opool.tile([P, MC_W], _F16, name="ot")
                    last = (mc == N_MC - 1 and ui == len(U_CHUNKS) - 1)
                    if last:
                        # split the final copy+store so the first half's DMA
                        # overlaps the second half's copy
                        for h0 in (0, MC_W // 2):
                            hs = slice(h0, h0 + MC_W // 2)
                            nc.vector.tensor_copy(ot[0:uw, hs], acc[0:uw, hs])
                            nc.scalar.dma_start(
                                out_d[u0:u0 + uw,
                                      mc * MC_W + h0:mc * MC_W + h0 + MC_W // 2],
                                ot[0:uw, hs])
                    else:
                        nc.vector.tensor_copy(ot[0:uw, :], acc[0:uw, :])
                        nc.scalar.dma_start(out_d[u0:u0 + uw, ms], ot[0:uw, :])

    nc.compile()
    return nc


def _fold_weights(trend_w, seasonal_w, trend_b, seasonal_b):
    """W = seasonal_w + (trend_w - seasonal_w) @ A via the banded structure of
    A; returns [KE, U] = [W^T; b] ready for the device."""
    trend_w = np.asarray(trend_w, dtype=np.float64)
    seasonal_w = np.asarray(seasonal_w, dtype=np.float64)
    trend_b = np.asarray(trend_b, dtype=np.float64)
    seasonal_b = np.asarray(seasonal_b, dtype=np.float64)
    counts = np.minimum(np.arange(T) + 1, WINDOW).astype(np.float64)
    G = (trend_w - seasonal_w) / counts[None, :]
    M = np.zeros_like(G)
    for d in range(WINDOW):
        M[:, :T - d] += G[:, d:]
    W = seasonal_w + M
    b = trend_b + seasonal_b
    wt_ext = np.empty((KE, U), np.float32)
    wt_ext[:T, :] = W.T.astype(np.float32)
    wt_ext[T, :] = b.astype(np.float32)
    return wt_ext


_NC_CACHE = {}
RUN_KWARGS = {}   # test harness may set {"trace": True}
LAST_RESULTS = None


def kernel(x, trend_w, trend_b, seasonal_w, seasonal_b):
    global LAST_RESULTS
    wt16 = _fold_weights(trend_w, seasonal_w, trend_b,
                         seasonal_b).astype(np.float16)

    # Pre-transposed, ones-row-extended fp16 shards [721, 2048] per core.
    x2d = np.asarray(x, dtype=np.float32).reshape(M_TOT, T)
    xt_all = np.empty((KE, M_TOT), np.float16)
    xt_all[:T] = x2d.T.astype(np.float16)
    xt_all[T] = 1.0
    xt_cores = [np.ascontiguousarray(xt_all[:, i * M_LOC:(i + 1) * M_LOC])
                for i in range(N_CORES)]

    if "nc" not in _NC_CACHE:
        _NC_CACHE["nc"] = _build_nc()
    nc = _NC_CACHE["nc"]

    in_maps = [{"xt": xt_cores[i], "wt": wt16} for i in range(N_CORES)]
    res = run_bass_kernel_spmd(nc, in_maps, core_ids=list(range(N_CORES)),
                               **RUN_KWARGS)
    LAST_RESULTS = res
    out_t = np.concatenate([r["out"] for r in res.results], axis=1)
    return out_t.T.astype(np.float32).reshape(B, NPTS, U)
